# revision 13
# baseline (speedup 1.0000x reference)
"""Deformable convolution (DCNv1, 3x3, pad=1) on 8 Trainium2 NeuronCores.

Sharding: data-parallel over batch — one sample per core, weights replicated.

Per-core algorithm:
  1. Index/weight math on the vector engine from offsets (fp32; positions are
     pre-shifted +1 so all arithmetic is non-negative).
  2. One dma_gather descriptor per (tap, pixel) fetches the full 2x2 bilinear
     patch (512 fp16 values) from a row-pair-interleaved channels-last copy
     of the image in DRAM. Calls rotate over the 4 SWDGE queues.
  3. Bilinear blend in fp16 pixel-major layout: one broadcast-AP
     tensor_tensor multiply per corner + adds, 2048-wide ops.
  4. TensorE transpose (fp16, 1 cy/row) to channel-major im2col columns,
     4 transposes batched per PSUM bank before ACT evacuation.
  5. Conv = 9 accumulated fp16 matmuls into fp32 PSUM; bias on evacuation.

Numerics: gather/blend/cols/weights in fp16, PSUM accumulation fp32.
Empirical end-to-end rel err vs fp32 reference: ~6e-4.
"""
from contextlib import ExitStack

import numpy as np

import concourse.bass as bass
import concourse.bacc as bacc
import concourse.tile as tile
from concourse import mybir
from concourse.bass import AP
from concourse import library_config
from concourse.bass_utils import run_bass_kernel_spmd

F32 = mybir.dt.float32
F16 = mybir.dt.float16
I32 = mybir.dt.int32
I16 = mybir.dt.int16

KH = KW = 3
K = 9
H = W = 64
HW = H * W
C = 128
O = 128
PAD_PX = 65
NV = 4352
TOT_PX = 4480
GELEM = 512          # one 2x2 patch: [x00|x10|x01|x11], fp16
GSTEP = 256          # slot stride (one pixel-row-pair slot)
MAXDESC = 1024       # dma_gather descriptor-ring limit per call
NB = 32
CHUNKS = 4
NBC = NB // CHUNKS   # 16 blocks/chunk
PXC = HW // CHUNKS   # 2048 px/chunk

# corner order matches the gathered patch layout: slot ci = dx*2 + dy
CORNERS = ((0, 0), (1, 0), (0, 1), (1, 1))  # (dy, dx) for ci = 0..3


def _make_base_const() -> np.ndarray:
    p = np.arange(HW)
    py = (p // W).astype(np.float32)
    px = (p % W).astype(np.float32)
    base = np.empty((18, HW), np.float32)
    for ki in range(KH):
        for kj in range(KW):
            k = ki * KW + kj
            base[2 * k] = py + ki
            base[2 * k + 1] = px + kj
    return np.ascontiguousarray(base.reshape(18, NB, 128).transpose(2, 0, 1))


def _prep_core_inputs(x_b, offset_b, weight, bias, base_const) -> dict:
    xclb = np.zeros((TOT_PX + W, C), np.float16)
    xclb[PAD_PX:PAD_PX + HW] = x_b.reshape(C, HW).T.astype(np.float16)
    xcl = np.zeros((TOT_PX, 2 * C), np.float16)
    xcl[:, :C] = xclb[:TOT_PX]
    xcl[:, C:] = xclb[W:TOT_PX + W]
    offs = np.ascontiguousarray(
        offset_b.reshape(18, NB, 128).transpose(2, 0, 1)).astype(np.float32)
    wts = np.ascontiguousarray(
        weight.reshape(O, C, K).transpose(2, 1, 0)).astype(np.float16)
    return {
        "xcl": xcl,
        "offs": offs,
        "base": base_const,
        "wts": wts,
        "bias_in": bias.reshape(O, 1).astype(np.float32),
        "ident_in": np.eye(128, dtype=np.float16),
    }


def _bcast(ap, n):
    """Append a step-0 length-n innermost dim to an AP (free-dim broadcast)."""
    return bass.AP(tensor=ap.tensor, offset=ap.offset, ap=[*ap.ap, [0, n]])


def _dcn_core_kernel(tc, outs, ins):
    nc = tc.nc
    out_d = outs["out"]

    with ExitStack() as ctx:
        consts = ctx.enter_context(tc.tile_pool(name="consts", bufs=1))
        idxp = ctx.enter_context(tc.tile_pool(name="idx", bufs=1))
        gath = ctx.enter_context(tc.tile_pool(name="gath", bufs=3))
        pmp = ctx.enter_context(tc.tile_pool(name="pm", bufs=3))
        colp = ctx.enter_context(tc.tile_pool(name="col", bufs=2))
        outp = ctx.enter_context(tc.tile_pool(name="outsb", bufs=2))
        psums = ctx.enter_context(tc.tile_pool(name="psums", bufs=4, space="PSUM"))
        psumc = ctx.enter_context(tc.tile_pool(name="psumc", bufs=1, space="PSUM"))

        offs = consts.tile([128, K, 2, NB], F32)
        base = consts.tile([128, K, 2, NB], F32)
        nc.sync.dma_start(out=offs, in_=ins["offs"])
        nc.sync.dma_start(out=base, in_=ins["base"])
        wts = consts.tile([128, K, O], F16)
        for k in range(K):
            nc.sync.dma_start(out=wts[:, k, :], in_=ins["wts"][k])
        bias_sb = consts.tile([128, 1], F32)
        nc.sync.dma_start(out=bias_sb, in_=ins["bias_in"])
        ident = consts.tile([128, 128], F16)
        nc.sync.dma_start(out=ident, in_=ins["ident_in"])
        nc.gpsimd.load_library(library_config.mlp)

        # ---- index & weight math (fp32, [128, 576])
        pos = idxp.tile([128, K, 2, NB], F32)
        nc.vector.tensor_tensor(out=pos, in0=offs, in1=base, op=mybir.AluOpType.add)
        nc.vector.tensor_scalar(out=pos, in0=pos, scalar1=0.0, scalar2=65.0,
                                op0=mybir.AluOpType.max, op1=mybir.AluOpType.min)
        fi = idxp.tile([128, K, 2, NB], I32)
        nc.vector.tensor_copy(out=fi, in_=pos)
        fint = idxp.tile([128, K, 2, NB], F32)
        nc.vector.tensor_copy(out=fint, in_=fi)
        gt = idxp.tile([128, K, 2, NB], F32)
        nc.vector.tensor_tensor(out=gt, in0=fint, in1=pos, op=mybir.AluOpType.is_gt)
        nc.vector.tensor_tensor(out=fint, in0=fint, in1=gt,
                                op=mybir.AluOpType.subtract)
        frac = idxp.tile([128, K, 2, NB], F32)
        nc.vector.tensor_tensor(out=frac, in0=pos, in1=fint,
                                op=mybir.AluOpType.subtract)
        v0 = idxp.tile([128, K, 2, NB], F32)
        v1 = idxp.tile([128, K, 2, NB], F32)
        nc.vector.tensor_scalar(out=v0, in0=fint, scalar1=1.0, scalar2=None,
                                op0=mybir.AluOpType.is_ge)
        nc.vector.tensor_scalar(out=v1, in0=fint, scalar1=64.0, scalar2=None,
                                op0=mybir.AluOpType.is_le)
        nc.vector.tensor_tensor(out=v0, in0=v0, in1=v1, op=mybir.AluOpType.mult)
        nc.vector.tensor_scalar(out=v1, in0=fint, scalar1=63.0, scalar2=None,
                                op0=mybir.AluOpType.is_le)
        w0 = idxp.tile([128, K, 2, NB], F32)
        w1 = idxp.tile([128, K, 2, NB], F32)
        nc.vector.tensor_scalar(out=w0, in0=frac, scalar1=-1.0, scalar2=1.0,
                                op0=mybir.AluOpType.mult, op1=mybir.AluOpType.add)
        nc.vector.tensor_tensor(out=w0, in0=w0, in1=v0, op=mybir.AluOpType.mult)
        nc.vector.tensor_tensor(out=w1, in0=frac, in1=v1, op=mybir.AluOpType.mult)
        # fp16 corner weights, slot order ci = dx*2 + dy
        w4 = idxp.tile([128, K, 4, NB], F16)
        wy = (w0, w1)
        wx = (w0, w1)
        for ci, (dy, dx) in enumerate(CORNERS):
            nc.vector.tensor_tensor(
                out=w4[:, :, ci, :], in0=wy[dy][:, :, 0, :], in1=wx[dx][:, :, 1, :],
                op=mybir.AluOpType.mult)
        gidx_f = idxp.tile([128, K, NB], F32)
        nc.vector.tensor_scalar(out=gidx_f, in0=fint[:, :, 0, :], scalar1=64.0,
                                scalar2=None, op0=mybir.AluOpType.mult)
        nc.vector.tensor_tensor(out=gidx_f, in0=gidx_f, in1=fint[:, :, 1, :],
                                op=mybir.AluOpType.add)
        gidx16 = idxp.tile([128, K * NB], I16)
        nc.vector.tensor_copy(out=gidx16, in_=gidx_f[:, :, :])

        # wrap-16 indices per tap for dma_gather: idx j=b*128+q at
        # (q%16, b*8 + q//16), staged into the partition group of the
        # SWDGE queue that tap's gathers run on (cores 2q, 2q+1 read
        # partitions [32q, 32q+32)). Per-tap tiles keep the gathers'
        # dependencies fine-grained so tap 0 can start early.
        # indices wrapped in 16 partitions, replicated to all 8 core groups
        NC1 = K * NB
        idxw = idxp.tile([128, NC1 * 8], I16)
        for qh in range(8):
            s = gidx16[qh * 16:(qh + 1) * 16, :]
            d0 = idxw[0:16, :]
            d = bass.AP(tensor=d0.tensor, offset=d0.offset + qh,
                        ap=[d0.ap[0], [8, NC1]])
            nc.sync.dma_start(out=d, in_=s)
        for g in range(1, 8):
            nc.sync.dma_start(out=idxw[16 * g:16 * (g + 1), :], in_=idxw[0:16, :])

        xview = AP(tensor=ins["xcl"].tensor, offset=0,
                   ap=[[GSTEP, NV], [1, GELEM]])

        for ch in range(CHUNKS):
            conv_ps = psumc.tile([128, PXC], F32, space="PSUM")
            bs = ch * NBC
            for k in range(K):
                gk = gath.tile([128, NBC, GELEM], F16)
                c0 = (k * NB + ch * NBC) * 8
                nc.gpsimd.dma_gather(
                    out_ap=gk[:, :, :],
                    in_ap=xview,
                    idxs_ap=idxw[:, c0:c0 + NBC * 8],
                    num_idxs=NBC * 128,
                    num_idxs_reg=NBC * 128,
                    elem_size=GELEM,
                    elem_step=GSTEP,
                    queue_num=(ch * K + k) % 4,
                )
                # weighted-diagonal moving operands: Dk[q, ci, b, j] =
                # ident[q, j] * w4[q, k, ci, bs+b]. One broadcast-AP multiply;
                # the corner SUM then rides the PE's fp32 PSUM accumulation,
                # so fp16 rounding only touches the inputs, not the blend.
                dk = pmp.tile([128, 4, NBC, C], F16)
                i0 = ident[:, :]
                ident_b = bass.AP(tensor=i0.tensor, offset=i0.offset,
                                  ap=[i0.ap[0], [0, 4], [0, NBC], [1, C]])
                wv = w4[:, k, :, bs:bs + NBC]
                w_b = bass.AP(tensor=wv.tensor, offset=wv.offset,
                              ap=[wv.ap[0], wv.ap[1], wv.ap[2], [0, C]])
                nc.vector.tensor_tensor(out=dk[:, :, :, :], in0=ident_b, in1=w_b,
                                        op=mybir.AluOpType.mult)
                # per pixel block: psum[c, j] += sum_ci gk_ci.T @ diag(w_ci)
                colk = colp.tile([128, PXC], F16)
                for bg in range(NBC // 4):
                    pst = psums.tile([128, 512], F32, space="PSUM")
                    for j in range(4):
                        b = bg * 4 + j
                        for ci in range(4):
                            nc.tensor.matmul(
                                out=pst[:, j * 128:(j + 1) * 128],
                                lhsT=gk[:, b, ci * C:(ci + 1) * C],
                                rhs=dk[:, ci, b, :],
                                start=(ci == 0), stop=(ci == 3))
                    nc.scalar.copy(out=colk[:, bg * 512:(bg + 1) * 512], in_=pst)
                for m in range(PXC // 512):
                    nc.tensor.matmul(
                        out=conv_ps[:, m * 512:(m + 1) * 512],
                        lhsT=wts[:, k, :],
                        rhs=colk[:, m * 512:(m + 1) * 512],
                        start=(k == 0), stop=(k == K - 1))
            out_sb = outp.tile([128, PXC], F32)
            nc.scalar.activation(out=out_sb, in_=conv_ps,
                                 func=mybir.ActivationFunctionType.Identity,
                                 bias=bias_sb[:, :], scale=1.0)
            nc.sync.dma_start(out=out_d[:, ch * PXC:(ch + 1) * PXC], in_=out_sb)


_IN_SPECS = {
    "xcl": ((TOT_PX, 2 * C), np.float16),
    "offs": ((128, 18, NB), np.float32),
    "base": ((128, 18, NB), np.float32),
    "wts": ((K, C, O), np.float16),
    "bias_in": ((O, 1), np.float32),
    "ident_in": ((128, 128), np.float16),
}

_prog_cache = {}


def _build_program():
    if "nc" in _prog_cache:
        return _prog_cache["nc"]
    nc = bacc.Bacc("TRN2", target_bir_lowering=False, debug=False,
                   num_swdge_queues=4)
    ins = {}
    for name, (shape, dtype) in _IN_SPECS.items():
        ins[name] = nc.dram_tensor(
            name, list(shape), mybir.dt.from_np(np.dtype(dtype)),
            kind="ExternalInput").ap()
    outs = {"out": nc.dram_tensor("out", [O, HW], F32,
                                  kind="ExternalOutput").ap()}
    with tile.TileContext(nc) as tc:
        _dcn_core_kernel(tc, outs, ins)
    nc.compile()
    _prog_cache["nc"] = nc
    return nc


def run_dcn(x, offset, weight, bias, trace=False):
    x = np.ascontiguousarray(x, dtype=np.float32)
    offset = np.ascontiguousarray(offset, dtype=np.float32)
    weight = np.ascontiguousarray(weight, dtype=np.float32)
    bias = np.ascontiguousarray(bias, dtype=np.float32)
    B = x.shape[0]
    base_const = _make_base_const()
    in_maps = [_prep_core_inputs(x[b], offset[b], weight, bias, base_const)
               for b in range(B)]
    nc = _build_program()
    res = run_bass_kernel_spmd(nc, in_maps, core_ids=list(range(B)), trace=trace)
    out = np.stack([r["out"] for r in res.results]).reshape(B, O, H, W)
    return out, res


def kernel(x, offset, weight, bias):
    out, _ = run_dcn(x, offset, weight, bias)
    return out.astype(np.float32)


# revision 14
# speedup vs baseline: 1.1167x; 1.1167x over previous
"""Deformable convolution (DCNv1, 3x3, pad=1) on 8 Trainium2 NeuronCores.

Sharding: data-parallel over batch — one sample per core, weights replicated.

Per-core algorithm:
  1. Index/weight math on the vector engine from offsets (fp32; positions are
     pre-shifted +1 so all arithmetic is non-negative).
  2. One dma_gather descriptor per (tap, pixel) fetches the full 2x2 bilinear
     patch (512 fp16 values) from a row-pair-interleaved channels-last copy
     of the image in DRAM. Calls rotate over the 4 SWDGE queues.
  3. Bilinear blend in fp16 pixel-major layout: one broadcast-AP
     tensor_tensor multiply per corner + adds, 2048-wide ops.
  4. TensorE transpose (fp16, 1 cy/row) to channel-major im2col columns,
     4 transposes batched per PSUM bank before ACT evacuation.
  5. Conv = 9 accumulated fp16 matmuls into fp32 PSUM; bias on evacuation.

Numerics: gather/blend/cols/weights in fp16, PSUM accumulation fp32.
Empirical end-to-end rel err vs fp32 reference: ~6e-4.
"""
from contextlib import ExitStack

import numpy as np

import concourse.bass as bass
import concourse.bacc as bacc
import concourse.tile as tile
from concourse import mybir
from concourse.bass import AP
from concourse import library_config
from concourse.bass_utils import run_bass_kernel_spmd

F32 = mybir.dt.float32
F16 = mybir.dt.float16
I32 = mybir.dt.int32
I16 = mybir.dt.int16

KH = KW = 3
K = 9
H = W = 64
HW = H * W
C = 128
O = 128
PAD_PX = 65
NV = 4352
TOT_PX = 4480
GELEM = 512          # one 2x2 patch: [x00|x10|x01|x11], fp16
GSTEP = 256          # slot stride (one pixel-row-pair slot)
MAXDESC = 1024       # dma_gather descriptor-ring limit per call
NB = 32
CHUNKS = 2
NBC = NB // CHUNKS   # 16 blocks/chunk
PXC = HW // CHUNKS   # 2048 px/chunk

# corner order matches the gathered patch layout: slot ci = dx*2 + dy
CORNERS = ((0, 0), (1, 0), (0, 1), (1, 1))  # (dy, dx) for ci = 0..3


def _make_base_const() -> np.ndarray:
    p = np.arange(HW)
    py = (p // W).astype(np.float32)
    px = (p % W).astype(np.float32)
    base = np.empty((18, HW), np.float32)
    for ki in range(KH):
        for kj in range(KW):
            k = ki * KW + kj
            base[2 * k] = py + ki
            base[2 * k + 1] = px + kj
    return np.ascontiguousarray(base.reshape(18, NB, 128).transpose(2, 0, 1))


def _prep_core_inputs(x_b, offset_b, weight, bias, base_const) -> dict:
    xclb = np.zeros((TOT_PX + W, C), np.float16)
    xclb[PAD_PX:PAD_PX + HW] = x_b.reshape(C, HW).T.astype(np.float16)
    xcl = np.zeros((TOT_PX, 2 * C), np.float16)
    xcl[:, :C] = xclb[:TOT_PX]
    xcl[:, C:] = xclb[W:TOT_PX + W]
    offs = np.ascontiguousarray(
        offset_b.reshape(18, NB, 128).transpose(2, 0, 1)).astype(np.float32)
    wts = np.ascontiguousarray(
        weight.reshape(O, C, K).transpose(2, 1, 0)).astype(np.float16)
    return {
        "xcl": xcl,
        "offs": offs,
        "base": base_const,
        "wts": wts,
        "bias_in": bias.reshape(O, 1).astype(np.float32),
        "ident_in": np.eye(128, dtype=np.float16),
    }


def _bcast(ap, n):
    """Append a step-0 length-n innermost dim to an AP (free-dim broadcast)."""
    return bass.AP(tensor=ap.tensor, offset=ap.offset, ap=[*ap.ap, [0, n]])


def _dcn_core_kernel(tc, outs, ins):
    nc = tc.nc
    out_d = outs["out"]

    with ExitStack() as ctx:
        consts = ctx.enter_context(tc.tile_pool(name="consts", bufs=1))
        idxp = ctx.enter_context(tc.tile_pool(name="idx", bufs=1))
        gath = ctx.enter_context(tc.tile_pool(name="gath", bufs=3))
        pmp = ctx.enter_context(tc.tile_pool(name="pm", bufs=3))
        colp = ctx.enter_context(tc.tile_pool(name="col", bufs=2))
        outp = ctx.enter_context(tc.tile_pool(name="outsb", bufs=2))
        psums = ctx.enter_context(tc.tile_pool(name="psums", bufs=4, space="PSUM"))
        psumc = ctx.enter_context(tc.tile_pool(name="psumc", bufs=1, space="PSUM"))

        offs = consts.tile([128, K, 2, NB], F32)
        base = consts.tile([128, K, 2, NB], F32)
        nc.sync.dma_start(out=offs, in_=ins["offs"])
        nc.sync.dma_start(out=base, in_=ins["base"])
        wts = consts.tile([128, K, O], F16)
        for k in range(K):
            nc.sync.dma_start(out=wts[:, k, :], in_=ins["wts"][k])
        bias_sb = consts.tile([128, 1], F32)
        nc.sync.dma_start(out=bias_sb, in_=ins["bias_in"])
        ident = consts.tile([128, 128], F16)
        nc.sync.dma_start(out=ident, in_=ins["ident_in"])
        nc.gpsimd.load_library(library_config.mlp)

        # ---- index & weight math (fp32, [128, 576])
        pos = idxp.tile([128, K, 2, NB], F32)
        nc.vector.tensor_tensor(out=pos, in0=offs, in1=base, op=mybir.AluOpType.add)
        nc.vector.tensor_scalar(out=pos, in0=pos, scalar1=0.0, scalar2=65.0,
                                op0=mybir.AluOpType.max, op1=mybir.AluOpType.min)
        fi = idxp.tile([128, K, 2, NB], I32)
        nc.vector.tensor_copy(out=fi, in_=pos)
        fint = idxp.tile([128, K, 2, NB], F32)
        nc.vector.tensor_copy(out=fint, in_=fi)
        gt = idxp.tile([128, K, 2, NB], F32)
        nc.vector.tensor_tensor(out=gt, in0=fint, in1=pos, op=mybir.AluOpType.is_gt)
        nc.vector.tensor_tensor(out=fint, in0=fint, in1=gt,
                                op=mybir.AluOpType.subtract)
        frac = idxp.tile([128, K, 2, NB], F32)
        nc.vector.tensor_tensor(out=frac, in0=pos, in1=fint,
                                op=mybir.AluOpType.subtract)
        v0 = idxp.tile([128, K, 2, NB], F32)
        v1 = idxp.tile([128, K, 2, NB], F32)
        nc.vector.tensor_scalar(out=v0, in0=fint, scalar1=1.0, scalar2=None,
                                op0=mybir.AluOpType.is_ge)
        nc.vector.tensor_scalar(out=v1, in0=fint, scalar1=64.0, scalar2=None,
                                op0=mybir.AluOpType.is_le)
        nc.vector.tensor_tensor(out=v0, in0=v0, in1=v1, op=mybir.AluOpType.mult)
        nc.vector.tensor_scalar(out=v1, in0=fint, scalar1=63.0, scalar2=None,
                                op0=mybir.AluOpType.is_le)
        w0 = idxp.tile([128, K, 2, NB], F32)
        w1 = idxp.tile([128, K, 2, NB], F32)
        nc.vector.tensor_scalar(out=w0, in0=frac, scalar1=-1.0, scalar2=1.0,
                                op0=mybir.AluOpType.mult, op1=mybir.AluOpType.add)
        nc.vector.tensor_tensor(out=w0, in0=w0, in1=v0, op=mybir.AluOpType.mult)
        nc.vector.tensor_tensor(out=w1, in0=frac, in1=v1, op=mybir.AluOpType.mult)
        # fp16 corner weights, slot order ci = dx*2 + dy
        w4 = idxp.tile([128, K, 4, NB], F16)
        wy = (w0, w1)
        wx = (w0, w1)
        for ci, (dy, dx) in enumerate(CORNERS):
            nc.vector.tensor_tensor(
                out=w4[:, :, ci, :], in0=wy[dy][:, :, 0, :], in1=wx[dx][:, :, 1, :],
                op=mybir.AluOpType.mult)
        gidx_f = idxp.tile([128, K, NB], F32)
        nc.vector.tensor_scalar(out=gidx_f, in0=fint[:, :, 0, :], scalar1=64.0,
                                scalar2=None, op0=mybir.AluOpType.mult)
        nc.vector.tensor_tensor(out=gidx_f, in0=gidx_f, in1=fint[:, :, 1, :],
                                op=mybir.AluOpType.add)
        gidx16 = idxp.tile([128, K * NB], I16)
        nc.vector.tensor_copy(out=gidx16, in_=gidx_f[:, :, :])

        # wrap-16 indices per tap for dma_gather: idx j=b*128+q at
        # (q%16, b*8 + q//16), staged into the partition group of the
        # SWDGE queue that tap's gathers run on (cores 2q, 2q+1 read
        # partitions [32q, 32q+32)). Per-tap tiles keep the gathers'
        # dependencies fine-grained so tap 0 can start early.
        # indices wrapped in 16 partitions, replicated to all 8 core groups
        NC1 = K * NB
        idxw = idxp.tile([128, NC1 * 8], I16)
        # alternate the two HWDGE queues so these small strided writes
        # drain in parallel instead of serializing on one ring
        for qh in range(8):
            s = gidx16[qh * 16:(qh + 1) * 16, :]
            d0 = idxw[0:16, :]
            d = bass.AP(tensor=d0.tensor, offset=d0.offset + qh,
                        ap=[d0.ap[0], [8, NC1]])
            eng = nc.sync if qh % 2 == 0 else nc.scalar
            eng.dma_start(out=d, in_=s)
        for g in range(1, 8):
            eng = nc.sync if g % 2 == 0 else nc.scalar
            eng.dma_start(out=idxw[16 * g:16 * (g + 1), :], in_=idxw[0:16, :])

        xview = AP(tensor=ins["xcl"].tensor, offset=0,
                   ap=[[GSTEP, NV], [1, GELEM]])

        for ch in range(CHUNKS):
            conv_ps = psumc.tile([128, PXC], F32, space="PSUM")
            bs = ch * NBC
            for k in range(K):
                gk = gath.tile([128, NBC, GELEM], F16)
                c0 = (k * NB + ch * NBC) * 8
                nblk = MAXDESC // 128
                for s in range(NBC // nblk):
                    nc.gpsimd.dma_gather(
                        out_ap=gk[:, s * nblk:(s + 1) * nblk, :],
                        in_ap=xview,
                        idxs_ap=idxw[:, c0 + s * nblk * 8:c0 + (s + 1) * nblk * 8],
                        num_idxs=nblk * 128,
                        num_idxs_reg=nblk * 128,
                        elem_size=GELEM,
                        elem_step=GSTEP,
                        queue_num=(k + ch) % 4,
                    )
                # weighted-diagonal moving operands: Dk[q, ci, b, j] =
                # ident[q, j] * w4[q, k, ci, bs+b]. One broadcast-AP multiply;
                # the corner SUM then rides the PE's fp32 PSUM accumulation,
                # so fp16 rounding only touches the inputs, not the blend.
                dk = pmp.tile([128, 4, NBC, C], F16)
                i0 = ident[:, :]
                ident_b = bass.AP(tensor=i0.tensor, offset=i0.offset,
                                  ap=[i0.ap[0], [0, 4], [0, NBC], [1, C]])
                wv = w4[:, k, :, bs:bs + NBC]
                w_b = bass.AP(tensor=wv.tensor, offset=wv.offset,
                              ap=[wv.ap[0], wv.ap[1], wv.ap[2], [0, C]])
                nc.vector.tensor_tensor(out=dk[:, :, :, :], in0=ident_b, in1=w_b,
                                        op=mybir.AluOpType.mult)
                # per pixel block: psum[c, j] += sum_ci gk_ci.T @ diag(w_ci)
                colk = colp.tile([128, PXC], F16)
                for bg in range(NBC // 4):
                    pst = psums.tile([128, 512], F32, space="PSUM")
                    for j in range(4):
                        b = bg * 4 + j
                        for ci in range(4):
                            nc.tensor.matmul(
                                out=pst[:, j * 128:(j + 1) * 128],
                                lhsT=gk[:, b, ci * C:(ci + 1) * C],
                                rhs=dk[:, ci, b, :],
                                start=(ci == 0), stop=(ci == 3))
                    nc.scalar.copy(out=colk[:, bg * 512:(bg + 1) * 512], in_=pst)
                for m in range(PXC // 512):
                    nc.tensor.matmul(
                        out=conv_ps[:, m * 512:(m + 1) * 512],
                        lhsT=wts[:, k, :],
                        rhs=colk[:, m * 512:(m + 1) * 512],
                        start=(k == 0), stop=(k == K - 1))
            out_sb = outp.tile([128, PXC], F32)
            nc.scalar.activation(out=out_sb, in_=conv_ps,
                                 func=mybir.ActivationFunctionType.Identity,
                                 bias=bias_sb[:, :], scale=1.0)
            nc.sync.dma_start(out=out_d[:, ch * PXC:(ch + 1) * PXC], in_=out_sb)


_IN_SPECS = {
    "xcl": ((TOT_PX, 2 * C), np.float16),
    "offs": ((128, 18, NB), np.float32),
    "base": ((128, 18, NB), np.float32),
    "wts": ((K, C, O), np.float16),
    "bias_in": ((O, 1), np.float32),
    "ident_in": ((128, 128), np.float16),
}

_prog_cache = {}


def _build_program():
    if "nc" in _prog_cache:
        return _prog_cache["nc"]
    nc = bacc.Bacc("TRN2", target_bir_lowering=False, debug=False,
                   num_swdge_queues=4)
    ins = {}
    for name, (shape, dtype) in _IN_SPECS.items():
        ins[name] = nc.dram_tensor(
            name, list(shape), mybir.dt.from_np(np.dtype(dtype)),
            kind="ExternalInput").ap()
    outs = {"out": nc.dram_tensor("out", [O, HW], F32,
                                  kind="ExternalOutput").ap()}
    with tile.TileContext(nc) as tc:
        _dcn_core_kernel(tc, outs, ins)
    nc.compile()
    _prog_cache["nc"] = nc
    return nc


def run_dcn(x, offset, weight, bias, trace=False):
    x = np.ascontiguousarray(x, dtype=np.float32)
    offset = np.ascontiguousarray(offset, dtype=np.float32)
    weight = np.ascontiguousarray(weight, dtype=np.float32)
    bias = np.ascontiguousarray(bias, dtype=np.float32)
    B = x.shape[0]
    base_const = _make_base_const()
    in_maps = [_prep_core_inputs(x[b], offset[b], weight, bias, base_const)
               for b in range(B)]
    nc = _build_program()
    res = run_bass_kernel_spmd(nc, in_maps, core_ids=list(range(B)), trace=trace)
    out = np.stack([r["out"] for r in res.results]).reshape(B, O, H, W)
    return out, res


def kernel(x, offset, weight, bias):
    out, _ = run_dcn(x, offset, weight, bias)
    return out.astype(np.float32)
